# revision 14
# baseline (speedup 1.0000x reference)
"""Trainium2 Bass kernel for nn_LHC_63806034150143 (gnn_message_passing).

Factorization: the reference's per-frame KNN-mean + kernel-regression scan is a
chain of static linear operators (knn graph and kernel weights depend only on
shapes), so y_t = x0 @ B_t with B_t = (G^T)^t @ Kd precomputed on host.
Device pipeline per core (SPMD, 8 cores, frames sharded 6 per core):
  encoder (replicated, 4 images) -> x0 -> per frame: y_t matmul -> decoder
  (upsample-fused phase convs) -> sigmoid -> strided DMA out.
Convs are tap-packed matmuls: shifted copies of the input are laid across
partitions (built with overlapping-window DMAs) so K = taps*channels.
"""
import numpy as np

import concourse.bass as bass
import concourse.bacc as bacc
import concourse.mybir as mybir
import concourse.tile as tile
from concourse.bass_utils import run_bass_kernel_spmd

fp32 = mybir.dt.float32

B = 4            # batch
CH = 32          # particle channels
S = 32           # particle grid side
N = S * S        # particles
F = 47           # generated frames
NCORES = 8
FPC = 6          # frames per core (last core: 5 real + 1 dummy)
KNN = 8

# jnp.linspace(-1, 1, 32) bit-exact values (f32), hardcoded to reproduce the
# reference's KNN graph without importing jax.
_LIN_HEX = ("000080bfe07b6fbfbef75ebf9c734ebf7cef3dbf5c6b2dbf3ae71cbf18630cbf"
            "f0bdf7beafb5d6be6cadb5be29a594bed03967be4e2925be9031c6bd082104bd"
            "0021043d8431c63d4829253ece39673e28a5943e69adb53eacb5d63eefbdf73e"
            "18630c3f38e71c3f5a6b2d3f7cef3d3f9c734e3fbcf75e3fde7b6f3f0000803f")


def _make_knn():
    lin = np.frombuffer(bytes.fromhex(_LIN_HEX), dtype=np.float32)
    pos = np.stack([np.broadcast_to(lin[:, None], (S, S)),
                    np.broadcast_to(lin[None, :], (S, S))], 0).reshape(2, N)
    # XLA cpu computes gram = fma(p1_i, p1_j, round_f32(p0_i * p0_j));
    # emulate: f32 first product, exact-f64 second product, one final rounding.
    a32 = (pos[0][:, None] * pos[0][None, :]).astype(np.float32)
    b64 = pos[1].astype(np.float64)[:, None] * pos[1].astype(np.float64)[None, :]
    gram = (a32.astype(np.float64) + b64).astype(np.float32)
    knn = np.argsort(gram, axis=-1, kind="stable")[:, :KNN]
    return pos, knn


def _make_Bts():
    pos, knn = _make_knn()
    G = np.zeros((N, N), np.float64)
    for n in range(N):
        for k in knn[n]:
            G[n, k] += 1.0 / KNN
    kernel_scale = 1.0 / np.sqrt(S**2 + S**2)
    ref_pos = pos.reshape(2, S, S)
    dist = ((pos[:, :, None, None].astype(np.float64)
             - ref_pos[:, None, :, :].astype(np.float64))**2).sum(0)
    Kd = np.maximum(1.0 - kernel_scale * dist, 0.0).reshape(N, N)
    GT = np.ascontiguousarray(G.T)
    Bts = np.empty((NCORES * FPC, N, N), np.float32)  # padded to 48
    C = Kd
    for t in range(F):
        C = GT @ C
        Bts[t] = (0.25 * C).astype(np.float32)  # fold final sum-pool scale
    Bts[F] = 0.0  # dummy frame for core 7
    return Bts


def _phase_weights(w):
    # w (O,I,3,3) -> E (2,2,O,I,2,2): conv3x3(upsample2(y), reflect) ==
    # 2x2-interleave of 4 phase 2x2 convs on clamp-padded y.
    O, I = w.shape[:2]
    E = np.zeros((2, 2, O, I, 2, 2), np.float32)
    rmap = {0: (0, 1, 1), 1: (0, 0, 1)}
    for pr in range(2):
        for pc in range(2):
            for dy in range(3):
                for dx in range(3):
                    E[pr, pc, :, :, rmap[pr][dy], rmap[pc][dx]] += w[:, :, dy, dx]
    return E


def _pack_weights(inp):
    f = np.float32
    ew1 = np.asarray(inp["enc_w1"], f); eb1 = np.asarray(inp["enc_b1"], f)
    ew2 = np.asarray(inp["enc_w2"], f); eb2 = np.asarray(inp["enc_b2"], f)
    ew3 = np.asarray(inp["enc_w3"], f); eb3 = np.asarray(inp["enc_b3"], f)
    dw1 = np.asarray(inp["dec_w1"], f); db1 = np.asarray(inp["dec_b1"], f)
    dw2 = np.asarray(inp["dec_w2"], f); db2 = np.asarray(inp["dec_b2"], f)
    dw3 = np.asarray(inp["dec_w3"], f); db3 = np.asarray(inp["dec_b3"], f)

    ew1f = 2.0 * ew1                       # x*2-1 folded
    eb1f = eb1 - ew1.sum((1, 2, 3))
    ew3f = 0.25 * ew3                      # pool1 scale folded

    d = {}
    # conv1e: lhsT (27, 32), row p = ch*9 + di*3 + dj
    e1 = np.zeros((27, 32), f)
    for ch in range(3):
        for di in range(3):
            for dj in range(3):
                e1[(di * 3 + dj) * 3 + ch, :] = ew1f[:, ch, di, dj]
    d["e1w"] = e1
    d["e1b"] = eb1f.reshape(32, 1)

    # conv2e: 3 matmuls (dj = m), lhsT (96, 3, 64), row p = di*32 + ch
    e2 = np.zeros((96, 3, 64), f)
    for m in range(3):
        for di in range(3):
            for ch in range(32):
                e2[di * 32 + ch, m, :] = ew2[:, ch, di, m]
    d["e2w"] = e2
    d["e2b"] = eb2.reshape(64, 1)

    # conv3e / conv2d (in 64ch, 5-stream): lhsT (128, 4, O) rows p = s*64+ch
    #   m=0..2 (dr=m): tap (m, s);  m=3: tap (s, 2);  single (64, O): tap (2,2)
    def pack5(w, O):
        a = np.zeros((128, 4, O), f)
        for s in range(2):
            for ch in range(64):
                for m in range(3):
                    a[s * 64 + ch, m, :] = w[:, ch, m, s]
                a[s * 64 + ch, 3, :] = w[:, ch, s, 2]
        single = np.ascontiguousarray(w[:, :, 2, 2].T)  # (64ch, O)
        return a, single

    d["e3w"], d["e3ws"] = pack5(ew3f, 32)
    d["e3b"] = eb3.reshape(32, 1)
    d["d2w"], d["d2ws"] = pack5(dw2, 32)
    d["d2b"] = db2.reshape(32, 1)

    # conv1d / conv3d phase convs: lhsT (128, 4, O), rows p = g*32+ch,
    # g = dr*2+dc, phase index q = pr*2+pc
    def packphase(E, O):
        a = np.zeros((128, 4, O), f)
        for pr in range(2):
            for pc in range(2):
                for g, (dr, dc) in enumerate([(0, 0), (0, 1), (1, 0), (1, 1)]):
                    for ch in range(E.shape[3]):
                        a[g * 32 + ch, pr * 2 + pc, :] = E[pr, pc, :, ch, dr, dc]
        return a

    d["d1w"] = packphase(_phase_weights(dw1), 64)
    d["d1b"] = db1.reshape(64, 1)
    d["d3w"] = packphase(_phase_weights(dw3), 3)
    d["d3b"] = (2.0 * db3).reshape(3, 1)   # sigmoid(2z) = (tanh(z)+1)/2
    d["ident"] = np.eye(32, dtype=f)
    return d


def _custom_ap(ap, dims, extra_offset=0):
    """Clone ap with custom [step, count] dims (first = partition dim)."""
    import copy
    b = copy.copy(ap)
    b.ap = mybir.VecI64Pair([list(p) for p in dims])
    b.offset = ap.offset + extra_offset
    return b


def build_program(nframes=FPC, nimages=B, nenc=B, debug_x0=False):
    """Build the SPMD Bass program. nframes/nimages/nenc shrink the program
    for simulation smoke tests (nenc = encoder images; must be B for real)."""
    nc = bacc.Bacc(None)

    xin = nc.dram_tensor("x", [B, 3, 128, 128], fp32, kind="ExternalInput")
    btin = nc.dram_tensor("bt", [FPC, N, N], fp32, kind="ExternalInput")
    wd = {}
    for nm, shp in [("e1w", [27, 32]), ("e1b", [32, 1]),
                    ("e2w", [96, 3, 64]), ("e2b", [64, 1]),
                    ("e3w", [128, 4, 32]), ("e3ws", [64, 32]), ("e3b", [32, 1]),
                    ("d1w", [128, 4, 64]), ("d1b", [64, 1]),
                    ("d2w", [128, 4, 32]), ("d2ws", [64, 32]), ("d2b", [32, 1]),
                    ("d3w", [128, 4, 3]), ("d3b", [3, 1]),
                    ("ident", [32, 32])]:
        wd[nm] = nc.dram_tensor(nm, shp, fp32, kind="ExternalInput")
    outd = nc.dram_tensor("out", [FPC, B, 3, 128, 128], fp32,
                          kind="ExternalOutput")
    x0dbg = nc.dram_tensor("x0dbg", [128, 8, 128], fp32,
                           kind="ExternalOutput") if debug_x0 else None

    AL = mybir.AluOpType
    ACT = mybir.ActivationFunctionType

    with tile.TileContext(nc) as tc:
        # ---- persistent pools -------------------------------------------
        with tc.tile_pool(name="const", bufs=1) as cpool, \
             tc.tile_pool(name="x0Tp", bufs=1) as x0p:
            w = {}
            for nm, t in wd.items():
                if nm in ("e3ws", "d2ws"):
                    # single-tap lhsT must live at base partition 64 to match
                    # its rhs (D2 upper half)
                    full = cpool.tile([128] + list(t.shape)[1:], fp32,
                                      tag=f"w_{nm}", name=f"w_{nm}")
                    nc.sync.dma_start(full[64:128], t[:])
                    w[nm] = full[64:128]
                else:
                    w[nm] = cpool.tile(list(t.shape), fp32, tag=f"w_{nm}",
                                       name=f"w_{nm}")
                    nc.sync.dma_start(w[nm][:], t[:])
            x0T = x0p.tile([128, 8, 128], fp32, tag="x0T")  # [n_loc, k, bc]

            # ================= ENCODER (replicated) =======================
            with tc.tile_pool(name="encs", bufs=1) as ep, \
                 tc.tile_pool(name="encps", bufs=2, space="PSUM") as epp, \
                 tc.tile_pool(name="tps", bufs=1, space="PSUM") as tpp:
                pT = tpp.tile([128, 8, 32 * B], fp32, tag="pT")
                for b in range(nenc):
                    # c1out: conv1e output, zero ring for conv2e (32,130,130)
                    c1out = ep.tile([32, 130, 130], fp32, tag="c1out")
                    nc.gpsimd.memset(c1out[:, 0, :], 0.0)
                    nc.gpsimd.memset(c1out[:, 129, :], 0.0)
                    nc.gpsimd.memset(c1out[:, 1:129, 0], 0.0)
                    nc.gpsimd.memset(c1out[:, 1:129, 129], 0.0)
                    for band in range(4):
                        r0 = band * 32
                        # ---- conv1e: reflect-padded band of x ------------
                        xb = ep.tile([3, 34, 130], fp32, tag="xband")
                        rlo, rhi = r0 - 1, r0 + 32      # image rows incl.
                        mlo, mhi = max(rlo, 0), min(rhi, 127)
                        nc.sync.dma_start(
                            xb[:, mlo - rlo:mhi - rlo + 1, 1:129],
                            xin[b, :, mlo:mhi + 1, :])
                        if rlo < 0:    # reflect row -1 -> row 1
                            nc.sync.dma_start(xb[:, 0, 1:129], xin[b, :, 1, :])
                        if rhi > 127:  # reflect row 128 -> row 126
                            nc.sync.dma_start(xb[:, 33, 1:129], xin[b, :, 126, :])
                        nc.vector.tensor_copy(xb[:, :, 0], xb[:, :, 2])
                        nc.vector.tensor_copy(xb[:, :, 129], xb[:, :, 127])
                        # dup (27, 32, 128): [(tap,ch)][r,c]=xb[ch][r+di,c+dj]
                        c1d = ep.tile([27, 32, 128], fp32, tag="c1dup")
                        for di in range(3):
                            for dj in range(3):
                                p0 = (di * 3 + dj) * 3
                                nc.sync.dma_start(
                                    c1d[p0:p0 + 3],
                                    xb[:, di:di + 32, dj:dj + 128])
                        for k in range(8):
                            ps = epp.tile([32, 4, 128], fp32, tag="eps1")
                            nc.tensor.matmul(ps[:], w["e1w"][:],
                                             c1d[:, 4 * k:4 * k + 4, :])
                            nc.scalar.activation(
                                c1out[:, 1 + r0 + 4 * k:1 + r0 + 4 * k + 4, 1:129],
                                ps[:], ACT.Relu, bias=w["e1b"][:])
                    # ---- conv2e + pool1 -> D2e lower (c3e input) ---------
                    # D2e doubles as conv3e input (64, 66, 66), reflect ring
                    D2e = ep.tile([128, 66, 66], fp32, tag="D2e")
                    for band in range(4):
                        r0 = band * 32
                        e2d = ep.tile([96, 32, 130], fp32, tag="e2dup")
                        for di in range(3):
                            nc.sync.dma_start(e2d[di * 32:di * 32 + 32],
                                              c1out[:, r0 + di:r0 + di + 32, :])
                        for k in range(8):
                            c2b = ep.tile([64, 4, 128], fp32, tag="c2band")
                            ps = epp.tile([64, 4, 128], fp32, tag="eps2")
                            for m in range(3):
                                nc.tensor.matmul(
                                    ps[:], w["e2w"][:, m, :],
                                    e2d[:, 4 * k:4 * k + 4, m:m + 128],
                                    start=(m == 0), stop=(m == 2))
                            nc.scalar.activation(c2b[:], ps[:], ACT.Relu,
                                                 bias=w["e2b"][:])
                            t1 = ep.tile([64, 4, 64], fp32, tag="poolt1")
                            nc.vector.tensor_add(t1[:], c2b[:, :, 0:128:2],
                                                 c2b[:, :, 1:128:2])
                            r1 = 1 + 16 * band + 2 * k
                            nc.vector.tensor_add(
                                D2e[0:64, r1:r1 + 2, 1:65],
                                t1[:, 0:4:2, :], t1[:, 1:4:2, :])
                    # reflect ring (cols then rows)
                    nc.vector.tensor_copy(D2e[0:64, 1:65, 0], D2e[0:64, 1:65, 2])
                    nc.vector.tensor_copy(D2e[0:64, 1:65, 65], D2e[0:64, 1:65, 63])
                    nc.vector.tensor_copy(D2e[0:64, 0, :], D2e[0:64, 2, :])
                    nc.vector.tensor_copy(D2e[0:64, 65, :], D2e[0:64, 63, :])
                    # ---- conv3e (5-stream) + pool2 -> x0img --------------
                    nc.sync.dma_start(D2e[64:128, 0:65, :], D2e[0:64, 1:66, :])
                    D1e = ep.tile([128, 66, 64], fp32, tag="D1e")
                    nc.sync.dma_start(D1e[0:64], D2e[0:64, :, 0:64])
                    nc.sync.dma_start(D1e[64:128], D2e[0:64, :, 1:65])
                    x0img = ep.tile([32, 32, 32], fp32, tag="x0img")
                    for k in range(8):
                        ps = epp.tile([32, 8, 64], fp32, tag="eps3")
                        for m in range(3):
                            nc.tensor.matmul(ps[:], w["e3w"][:, m, :],
                                             D1e[:, m + 8 * k:m + 8 * k + 8, :],
                                             start=(m == 0), stop=False)
                        nc.tensor.matmul(ps[:], w["e3w"][:, 3, :],
                                         D2e[:, 8 * k:8 * k + 8, 2:66],
                                         start=False, stop=False)
                        nc.tensor.matmul(ps[:], w["e3ws"][:],
                                         D2e[64:128, 1 + 8 * k:9 + 8 * k, 2:66],
                                         start=False, stop=True)
                        h3 = ep.tile([32, 8, 64], fp32, tag="h3c")
                        nc.scalar.activation(h3[:], ps[:], ACT.Relu,
                                             bias=w["e3b"][:])
                        t2 = ep.tile([32, 8, 32], fp32, tag="poolt2")
                        nc.vector.tensor_add(t2[:], h3[:, :, 0:64:2],
                                             h3[:, :, 1:64:2])
                        nc.vector.tensor_add(x0img[:, 4 * k:4 * k + 4, :],
                                             t2[:, 0:8:2, :], t2[:, 1:8:2, :])
                    # ---- transpose into pT[:, k, 32b:32b+32] -------------
                    x0f = x0img[:].rearrange("p a b -> p (a b)")
                    for k in range(8):
                        nc.tensor.transpose(pT[:, k, 32 * b:32 * b + 32],
                                            x0f[:, 128 * k:128 * k + 128],
                                            w["ident"][:])
                if nenc == B:
                    nc.vector.tensor_copy(x0T[:], pT[:])
                else:  # smoke-test: fill missing cols with zeros first
                    nc.vector.memset(x0T[:], 0.0)
                    nc.vector.tensor_copy(x0T[:, :, 0:32 * nenc],
                                          pT[:, :, 0:32 * nenc])
            if x0dbg is not None:
                nc.sync.dma_start(x0dbg[:], x0T[:])

            # ================== FRAMES + DECODER ==========================
            with tc.tile_pool(name="btp", bufs=3) as btp, \
                 tc.tile_pool(name="decs", bufs=2) as dp, \
                 tc.tile_pool(name="decb", bufs=1) as dpb, \
                 tc.tile_pool(name="ypp", bufs=1, space="PSUM") as ypp, \
                 tc.tile_pool(name="dps", bufs=2, space="PSUM") as dpp:
                for t in range(nframes):
                    pY = ypp.tile([128, 1024], fp32, tag="pY")
                    for k in range(8):
                        btile = btp.tile([128, 1024], fp32, tag="btile")
                        nc.sync.dma_start(btile[:],
                                          btin[t, 128 * k:128 * k + 128, :])
                        for j in range(2):
                            nc.tensor.matmul(pY[:, 512 * j:512 * j + 512],
                                             x0T[:, k, :],
                                             btile[:, 512 * j:512 * j + 512],
                                             start=(k == 0), stop=(k == 7))
                    ypad = dp.tile([128, 34, 34], fp32, tag="ypad")
                    nc.vector.tensor_copy(
                        ypad[:, 1:33, 1:33],
                        pY[:].rearrange("p (a b) -> p a b", b=32))
                    nc.vector.tensor_copy(ypad[:, 1:33, 0], ypad[:, 1:33, 1])
                    nc.vector.tensor_copy(ypad[:, 1:33, 33], ypad[:, 1:33, 32])
                    nc.vector.tensor_copy(ypad[:, 0, :], ypad[:, 1, :])
                    nc.vector.tensor_copy(ypad[:, 33, :], ypad[:, 32, :])
                    for b in range(nimages):
                        self_decode(nc, tc, dp, dpb, dpp, w, ypad, outd, t, b, AL, ACT)
    nc.compile()
    return nc


def self_decode(nc, tc, dp, dpb, dpp, w, ypad, outd, t, b, AL, ACT):
    """Decoder for one (frame, image): conv1d -> conv2d -> conv3d -> out."""
    # ---- conv1d (phase conv on y) --------------------------------------
    d1 = dp.tile([128, 33, 33], fp32, tag="dup1")
    for g, (dr, dc) in enumerate([(0, 0), (0, 1), (1, 0), (1, 1)]):
        nc.sync.dma_start(d1[32 * g:32 * g + 32],
                          ypad[32 * b:32 * b + 32, dr:dr + 33, dc:dc + 33])
    # D2c2 doubles as conv2d input (64, 66, 66) with zero ring
    D2 = dpb.tile([128, 66, 66], fp32, tag="D2c2")
    nc.gpsimd.memset(D2[0:64, 0, :], 0.0)
    nc.gpsimd.memset(D2[0:64, 65, :], 0.0)
    nc.gpsimd.memset(D2[0:64, 1:65, 0], 0.0)
    nc.gpsimd.memset(D2[0:64, 1:65, 65], 0.0)
    for q in range(4):          # phase q = pr*2+pc
        pr, pc = q // 2, q % 2
        for j in range(2):
            ps1 = dpp.tile([64, 16, 32], fp32, tag="ps1")
            nc.tensor.matmul(ps1[:], w["d1w"][:, q, :],
                             d1[:, pr + 16 * j:pr + 16 * j + 16, pc:pc + 32])
            out_v = D2[0:64, 1 + pr:65:2, 1 + pc:65:2][:, 16 * j:16 * j + 16, :]
            nc.vector.tensor_scalar(out_v, ps1[:], w["d1b"][:], 0.0,
                                    op0=AL.add, op1=AL.max)
    # ---- conv2d (5-stream) ---------------------------------------------
    nc.gpsimd.tensor_copy(D2[64:128, 0:65, :], D2[0:64, 1:66, :])
    D1 = dpb.tile([128, 66, 64], fp32, tag="D1c2")
    nc.sync.dma_start(D1[0:64], D2[0:64, :, 0:64])
    nc.sync.dma_start(D1[64:128], D2[0:64, :, 1:65])
    # pin3: conv3d input (32, 66, 66), clamp ring
    pin3 = dpb.tile([32, 66, 66], fp32, tag="pin3")
    for k in range(8):
        ps2 = dpp.tile([32, 8, 64], fp32, tag="ps2")
        for m in range(3):
            nc.tensor.matmul(ps2[:], w["d2w"][:, m, :],
                             D1[:, m + 8 * k:m + 8 * k + 8, :],
                             start=(m == 0), stop=False)
        nc.tensor.matmul(ps2[:], w["d2w"][:, 3, :],
                         D2[:, 8 * k:8 * k + 8, 2:66], start=False, stop=False)
        nc.tensor.matmul(ps2[:], w["d2ws"][:],
                         D2[64:128, 1 + 8 * k:9 + 8 * k, 2:66],
                         start=False, stop=True)
        nc.vector.tensor_scalar(pin3[:, 1 + 8 * k:9 + 8 * k, 1:65], ps2[:],
                                w["d2b"][:], 0.0, op0=AL.add, op1=AL.max)
    nc.vector.tensor_copy(pin3[:, 1:65, 0], pin3[:, 1:65, 1])
    nc.vector.tensor_copy(pin3[:, 1:65, 65], pin3[:, 1:65, 64])
    nc.vector.tensor_copy(pin3[:, 0, :], pin3[:, 1, :])
    nc.vector.tensor_copy(pin3[:, 65, :], pin3[:, 64, :])
    # ---- conv3d (phase conv) + sigmoid ---------------------------------
    d3 = dpb.tile([128, 65, 65], fp32, tag="dup3")
    for g, (dr, dc) in enumerate([(0, 0), (0, 1), (1, 0), (1, 1)]):
        nc.sync.dma_start(d3[32 * g:32 * g + 32],
                          pin3[:, dr:dr + 65, dc:dc + 65])
    obuf = dp.tile([128, 64, 64], fp32, tag="obuf")
    for q in range(4):
        pr, pc = q // 2, q % 2
        for k in range(8):
            ps3 = dpp.tile([3, 8, 64], fp32, tag="ps3")
            nc.tensor.matmul(ps3[:], w["d3w"][:, q, :],
                             d3[:, pr + 8 * k:pr + 8 * k + 8, pc:pc + 64])
            nc.scalar.activation(obuf[32 * q:32 * q + 3, 8 * k:8 * k + 8, :],
                                 ps3[:], ACT.Sigmoid, bias=w["d3b"][:],
                                 scale=2.0)
    for q in range(4):
        pr, pc = q // 2, q % 2
        nc.sync.dma_start(outd[t, b][:, pr:128:2, pc:128:2],
                          obuf[32 * q:32 * q + 3])


# --------------------------- host orchestration ------------------------------
_CACHE = {}


def kernel(**inputs):
    x = np.ascontiguousarray(np.asarray(inputs["x"], np.float32))
    if "consts" not in _CACHE:
        _CACHE["consts"] = _make_Bts()
    Bts = _CACHE["consts"]
    wpack = _pack_weights(inputs)
    if "nc" not in _CACHE:
        _CACHE["nc"] = build_program()
    nc = _CACHE["nc"]

    in_maps = []
    for j in range(NCORES):
        m = {"x": x, "bt": np.ascontiguousarray(Bts[j * FPC:(j + 1) * FPC])}
        m.update(wpack)
        in_maps.append(m)
    res = run_bass_kernel_spmd(nc, in_maps, list(range(NCORES)))
    _CACHE["last_results"] = res
    outs = res.results

    out = np.empty((B, 48, 3, 128, 128), np.float32)
    out[:, 0] = ((x * 2.0 - 1.0) + 1.0) / 2.0
    for j in range(NCORES):
        o = outs[j]["out"]  # (FPC, B, 3, 128, 128)
        for tt in range(FPC):
            f = 1 + j * FPC + tt
            if f <= F:
                out[:, f] = o[tt]
    return out
